# revision 17
# baseline (speedup 1.0000x reference)
"""Causal flash attention for Trainium2, sharded 2 heads/core over 8 cores.

Math per head: out = softmax_causal(Q K^T / sqrt(D)) @ V,  Q/K/V [S=2048, D=64] fp32.

Device layout (per core, heads h0=2c, h1=2c+1):
  qT   [128, 2048]  rows 64h+d = Q[h]^T        (D on partitions, both heads stacked)
  kT   [128, 2048]  same for K
  vaug [2, 128, 1040] vaug[h, p, 65*kc+d] = V[h, 128*kc+p, d], d=64 column is ones
  outT [2, 64, 2048]  out[h]^T (normalized)

Scores are computed transposed (S^T[k, q] = K_chunk @ Q^T) so no transposes are
needed anywhere: softmax denominator comes out of the PV matmul via the ones
column of vaug (psum row 64), and the final division broadcasts 1/denom across
partitions with gpsimd.partition_broadcast.

The kernel is paced by the Scalar (ACT) engine's exp over every causal score
(34816 columns/core at 1 elem/cycle/lane, 1.2 GHz). Structure keeps ACT
saturated at that floor:
  - scores land in psum [128,1024] tiles (h0 in bank A, h1 in bank B; the
    head-pair matmuls run concurrently on PE row groups 0-63/64-127), one exp
    instruction per tile covering both heads.
  - diagonal chunks (width w < 512) are END-aligned against the bank boundary
    (h0 at [512-w:512], h1 at [512:512+w]) so the exp covers 2w contiguous
    columns with zero gap waste; the two smallest diagonal chunks of each span
    share one tile, giving 36 ACTIVATEs and exactly S^2/2-worth of columns.
  - causal triangle masks run on the DVE (multiply by a precomputed mask) so
    the Scalar queue carries nothing but the exps.
  - spans run smallest-first (0,1,2,3); each span's normalization tail is
    emitted one span late so every tail except the last hides under the next
    span's exp stream, and the last tail is pipelined in column halves with
    its denominator copies on the then-idle scalar engine.
  - inputs arrive as six large DMAs spread over the sync/scalar/gpsimd
    queues; outputs leave as fp16 (host converts back to fp32).

HW-verified pitfalls honored here: custom DVE ops (reciprocal_approx_fast)
mis-read operands at nonzero partition offsets and from PSUM (denominator row
is first copied to a partition-0 SBUF tile); matmul PSUM output must be fp32
on TRN2; DVE instructions may read at most one PSUM operand.
"""

import os
import sys

import ml_dtypes
import numpy as np

sys.path.insert(0, "/opt/trn_rl_repo")

import concourse.bass as bass
import concourse.bacc as bacc
import concourse.mybir as mybir
import concourse.tile as tile
from concourse.bass_utils import run_bass_kernel_spmd

B, H, S, D = 1, 16, 2048, 64
N_CORES = 8
HEADS_PER_CORE = H // N_CORES  # 2
N_CHUNKS = S // 128  # 16 key chunks per head
N_SPANS = S // 512  # 4 query spans per head
F32 = mybir.dt.float32
BF16 = mybir.dt.bfloat16
F16 = mybir.dt.float16

_NC = None
_LAST_RESULTS = None


def _build_bass():
    nc = bacc.Bacc("TRN2", target_bir_lowering=False)
    qT = nc.declare_dram_parameter("qT", [128, S], F16, isOutput=False)
    kT = nc.declare_dram_parameter("kT", [128, S], F16, isOutput=False)
    vaug = nc.declare_dram_parameter("vaug", [2, 128, 65 * N_CHUNKS], BF16, isOutput=False)
    outT = nc.declare_dram_parameter("outT", [2, 64, S], F16, isOutput=True)

    with tile.TileContext(nc) as tc:
        with (
            tc.tile_pool(name="const", bufs=1) as const,
            tc.tile_pool(name="inbuf", bufs=1) as inbuf,
            tc.tile_pool(name="pbuf", bufs=6) as pbuf,
            tc.tile_pool(name="nbuf", bufs=2) as nbuf,
            tc.tile_pool(name="ps_s", bufs=2, space="PSUM") as ps_s,
            tc.tile_pool(name="ps_o", bufs=2, space="PSUM") as ps_o,
        ):

            # Input loads: few big DMAs. Only SP/gpsimd queues can issue DMAs
            # (Activation is the pacing engine - keep it clean). k0 + q3 gate
            # the first matmul; the rest stream in while span 3 computes.
            q0 = inbuf.tile([128, 512], F16, tag="q0", name="q0")
            nc.sync.dma_start(out=q0, in_=qT[:, 0:512])
            k0 = inbuf.tile([128, 512], F16, tag="k0", name="k0")
            nc.scalar.dma_start(out=k0, in_=kT[:, 0:512])
            v0 = inbuf.tile([128, 65 * N_CHUNKS], BF16, tag="v0", name="v0")
            nc.sync.dma_start(out=v0, in_=vaug[0])
            krest = inbuf.tile([128, 1536], F16, tag="kr", name="krest")
            nc.scalar.dma_start(out=krest, in_=kT[:, 512:2048])
            v1 = inbuf.tile([128, 65 * N_CHUNKS], BF16, tag="v1", name="v1")
            nc.gpsimd.dma_start(out=v1, in_=vaug[1])
            q123 = inbuf.tile([128, 1536], F16, tag="q123", name="q123")
            nc.sync.dma_start(out=q123, in_=qT[:, 512:2048])
            vsb = [v0, v1]

            mtri = const.tile([128, 128], BF16, tag="mtri", name="mtri")
            nc.gpsimd.memset(mtri, 1.0)
            nc.gpsimd.affine_select(
                out=mtri,
                in_=mtri,
                compare_op=mybir.AluOpType.is_ge,
                fill=0.0,
                base=0,
                pattern=[[1, 128]],
                channel_multiplier=-1,
            )

            def k_slice(h, kc):
                # kT chunk [64, 128] for head h: lhsT of the scores matmul.
                if kc < 4:
                    return k0[64 * h : 64 * h + 64, 128 * kc : 128 * kc + 128]
                c = 128 * kc - 512
                return krest[64 * h : 64 * h + 64, c : c + 128]

            def q_slice(h, qs, qe):
                if qe <= 512:
                    return q0[64 * h : 64 * h + 64, qs:qe]
                assert qs >= 512
                return q123[64 * h : 64 * h + 64, qs - 512 : qe - 512]

            def v_slice(h, kc):
                return vsb[h][:, 65 * kc : 65 * kc + 65]

            def emit_span(s, po, flush_tails):
                qs, qe = 512 * s, 512 * (s + 1)
                # Tiles: non-diag chunks kc<4s get one [128,1024] tile each
                # (h0 block [0:512] bank A, h1 [512:1024] bank B). Diagonal
                # chunks are end-aligned: h0 at [512-w:512], h1 at [512:512+w],
                # so the exp covers 2w contiguous columns with no gap waste.
                # The two smallest diagonal chunks (w=256,128) share one tile.
                tiles = [[kc] for kc in range(4 * s)]
                tiles.append([4 * s])      # diag w=512 (full block, needs mask)
                tiles.append([4 * s + 1])  # diag w=384
                tiles.append([4 * s + 2, 4 * s + 3])  # diag w=256,128 merged
                for tix, kcs in enumerate(tiles):
                    if tix == 1:
                        flush_tails()
                    diag = kcs[0] >= 4 * s
                    pg = ps_s.tile([128, 1024], F32, tag="pss", name=f"pg_{s}_{kcs[0]}")
                    pe2 = pbuf.tile([128, 1024], BF16, tag="pe", name=f"pe_{s}_{kcs[0]}")
                    ws = [qe - max(qs, 128 * kc) for kc in kcs]
                    tot = sum(ws)
                    blocks = []  # (h, kc, off, w)
                    o = 512 - tot
                    for kc, w in reversed(list(zip(kcs, ws))):
                        blocks.append((0, kc, o, w))
                        o += w
                    o = 512
                    for kc, w in zip(kcs, ws):
                        blocks.append((1, kc, o, w))
                        o += w
                    lo, hi = 512 - tot, 512 + tot
                    # Score matmuls: per kc the (h0, h1) pair runs concurrently
                    # on PE row groups 0-63 / 64-127, draining into banks A/B.
                    order = sorted(blocks, key=lambda b: (b[1], b[0]))
                    for h, kc, off, w in order:
                        qb = qe - w
                        nc.tensor.matmul(
                            pg[:, off : off + w],
                            k_slice(h, kc),
                            q_slice(h, qb, qe),
                            start=True,
                            stop=True,
                        )
                    # One exp for the whole tile (both heads, all its chunks).
                    nc.scalar.activation(
                        out=pe2[:, lo:hi],
                        in_=pg[:, lo:hi],
                        func=mybir.ActivationFunctionType.Exp,
                        scale=0.125,
                    )
                    # Causal masks for diagonal chunks: zero the strict upper
                    # triangle of each block's first 128 query columns (DVE).
                    if diag:
                        for h, kc, off, w in order:
                            nc.vector.tensor_mul(
                                out=pe2[:, off : off + 128],
                                in0=pe2[:, off : off + 128],
                                in1=mtri,
                            )
                    # PV accumulation.
                    nkc = 4 * s + 4
                    for h, kc, off, w in order:
                        qb = qe - w
                        nc.tensor.matmul(
                            po[h][:, qb - qs : qb - qs + w],
                            v_slice(h, kc),
                            pe2[:, off : off + w],
                            start=(kc == 0),
                            stop=(kc == nkc - 1),
                        )

            def emit_tail(s, h, po, final=False):
                qs = 512 * s
                if final:
                    # final span's tails are the only exposed ones: pipeline
                    # in column halves with the copy on the now-idle scalar.
                    for hh in range(2):
                        dn = nbuf.tile([1, 512], F32, tag=f"dn{hh}", name=f"dnf{hh}")
                        r = nbuf.tile([1, 512], F32, tag=f"r{hh}", name=f"rf{hh}")
                        rb = nbuf.tile([64, 512], F32, tag=f"rb{hh}", name=f"rbf{hh}")
                        o_sb = nbuf.tile([64, 512], F16, tag=f"o{hh}", name=f"of{hh}")
                        for a, b in ((0, 256), (256, 512)):
                            nc.scalar.copy(out=dn[:, a:b], in_=po[hh][64:65, a:b])
                            nc.vector.reciprocal_approx_fast(
                                out=r[:, a:b], in_=dn[:, a:b]
                            )
                            nc.gpsimd.partition_broadcast(rb[:, a:b], r[0:1, a:b])
                            nc.vector.tensor_mul(
                                out=o_sb[:, a:b],
                                in0=po[hh][0:64, a:b],
                                in1=rb[:, a:b],
                            )
                            nc.sync.dma_start(
                                out=outT[hh, :, qs + a : qs + b], in_=o_sb[:, a:b]
                            )
                    return
                # 1/denom: copy the psum denominator row to a partition-0 SBUF
                # tile (custom DVE ops mis-read at nonzero partition offsets),
                # fast approx reciprocal (~51 ULP), broadcast across
                # partitions on the otherwise-idle gpsimd, normalize on DVE
                # (PSUM x SBUF -> f16), store via sync. The final span's tails
                # are the only exposed ones: pipeline them in column halves
                # with the copy on the (now idle) scalar engine.
                dn = nbuf.tile([1, 512], F32, tag=f"dn{h}", name=f"dn{h}_{s}")
                r = nbuf.tile([1, 512], F32, tag=f"r{h}", name=f"r{h}_{s}")
                rb = nbuf.tile([64, 512], F32, tag=f"rb{h}", name=f"rb{h}_{s}")
                o_sb = nbuf.tile([64, 512], F16, tag=f"o{h}", name=f"o{h}_{s}")
                nc.vector.tensor_copy(out=dn, in_=po[h][64:65, :])
                nc.vector.reciprocal_approx_fast(out=r, in_=dn)
                nc.gpsimd.partition_broadcast(rb[:, :], r[0:1, :])
                nc.vector.tensor_mul(out=o_sb, in0=po[h][0:64, :], in1=rb)
                nc.sync.dma_start(out=outT[h, :, 512 * s : 512 * (s + 1)], in_=o_sb)

            pending = []

            def flush_tails():
                while pending:
                    ps, ppo = pending.pop(0)
                    for h in range(2):
                        emit_tail(ps, h, ppo)

            for s in (0, 1, 2, 3):
                po = [
                    ps_o.tile([65, 512], F32, tag=f"po{hh}", name=f"po{hh}_{s}")
                    for hh in range(2)
                ]
                emit_span(s, po, flush_tails)
                pending.append((s, po))
            while pending:
                ps, ppo = pending.pop(0)
                emit_tail(ps, None, ppo, final=True)

    nc.compile()
    return nc


def _get_nc():
    global _NC
    if _NC is None:
        _NC = _build_bass()
    return _NC


def kernel(q, k, v):
    global _LAST_RESULTS
    q = np.asarray(q, dtype=np.float32)
    k = np.asarray(k, dtype=np.float32)
    v = np.asarray(v, dtype=np.float32)
    assert q.shape == (B, H, S, D)

    in_maps = []
    for c in range(N_CORES):
        h0 = HEADS_PER_CORE * c
        qTh = np.ascontiguousarray(
            q[0, h0 : h0 + 2].transpose(0, 2, 1).reshape(128, S)
        ).astype(np.float16)
        kTh = np.ascontiguousarray(
            k[0, h0 : h0 + 2].transpose(0, 2, 1).reshape(128, S)
        ).astype(np.float16)
        va = np.ones((2, 128, N_CHUNKS, 65), dtype=np.float32)
        va[..., :64] = (
            v[0, h0 : h0 + 2].reshape(2, N_CHUNKS, 128, 64).transpose(0, 2, 1, 3)
        )
        va16 = va.reshape(2, 128, 65 * N_CHUNKS).astype(ml_dtypes.bfloat16)
        in_maps.append({"qT": qTh, "kT": kTh, "vaug": va16})

    nc = _get_nc()
    res = run_bass_kernel_spmd(nc, in_maps, core_ids=list(range(N_CORES)))
    _LAST_RESULTS = res

    out = np.empty((B, H, S, D), dtype=np.float32)
    for c in range(N_CORES):
        ot = res.results[c]["outT"].astype(np.float32)  # [2, 64, 2048] f16
        out[0, 2 * c] = ot[0].T
        out[0, 2 * c + 1] = ot[1].T
    return out


# revision 18
# speedup vs baseline: 1.0260x; 1.0260x over previous
"""Causal flash attention for Trainium2, sharded 2 heads/core over 8 cores.

Math per head: out = softmax_causal(Q K^T / sqrt(D)) @ V,  Q/K/V [S=2048, D=64] fp32.

Device layout (per core, heads h0=2c, h1=2c+1):
  qT   [128, 2048]  rows 64h+d = Q[h]^T        (D on partitions, both heads stacked)
  kT   [128, 2048]  same for K
  vaug [2, 128, 1040] vaug[h, p, 65*kc+d] = V[h, 128*kc+p, d], d=64 column is ones
  outT [2, 64, 2048]  out[h]^T (normalized)

Scores are computed transposed (S^T[k, q] = K_chunk @ Q^T) so no transposes are
needed anywhere: softmax denominator comes out of the PV matmul via the ones
column of vaug (psum row 64), and the final division broadcasts 1/denom across
partitions with gpsimd.partition_broadcast.

The kernel is paced by the Scalar (ACT) engine's exp over every causal score
(34816 columns/core at 1 elem/cycle/lane, 1.2 GHz). Structure keeps ACT
saturated at that floor:
  - scores land in psum [128,1024] tiles (h0 in bank A, h1 in bank B; the
    head-pair matmuls run concurrently on PE row groups 0-63/64-127), one exp
    instruction per tile covering both heads.
  - diagonal chunks (width w < 512) are END-aligned against the bank boundary
    (h0 at [512-w:512], h1 at [512:512+w]) so the exp covers 2w contiguous
    columns with zero gap waste; the two smallest diagonal chunks of each span
    share one tile, giving 36 ACTIVATEs and exactly S^2/2-worth of columns.
  - causal triangle masks run on the DVE (multiply by a precomputed mask) so
    the Scalar queue carries nothing but the exps.
  - spans run smallest-first (0,1,2,3); each span's normalization tail is
    emitted one span late so every tail except the last hides under the next
    span's exp stream, and the last tail is pipelined in column halves with
    its denominator copies on the then-idle scalar engine.
  - inputs arrive as six large DMAs spread over the sync/scalar/gpsimd
    queues; outputs leave as fp16 (host converts back to fp32).

HW-verified pitfalls honored here: custom DVE ops (reciprocal_approx_fast)
mis-read operands at nonzero partition offsets and from PSUM (denominator row
is first copied to a partition-0 SBUF tile); matmul PSUM output must be fp32
on TRN2; DVE instructions may read at most one PSUM operand.
"""

import os
import sys

import ml_dtypes
import numpy as np

sys.path.insert(0, "/opt/trn_rl_repo")

import concourse.bass as bass
import concourse.bacc as bacc
import concourse.mybir as mybir
import concourse.tile as tile
from concourse.bass_utils import run_bass_kernel_spmd

B, H, S, D = 1, 16, 2048, 64
N_CORES = 8
HEADS_PER_CORE = H // N_CORES  # 2
N_CHUNKS = S // 128  # 16 key chunks per head
N_SPANS = S // 512  # 4 query spans per head
F32 = mybir.dt.float32
BF16 = mybir.dt.bfloat16
F16 = mybir.dt.float16

_NC = None
_LAST_RESULTS = None


def _build_bass():
    nc = bacc.Bacc("TRN2", target_bir_lowering=False)
    qT = nc.declare_dram_parameter("qT", [128, S], F16, isOutput=False)
    kT = nc.declare_dram_parameter("kT", [128, S], F16, isOutput=False)
    vaug = nc.declare_dram_parameter("vaug", [2, 128, 65 * N_CHUNKS], BF16, isOutput=False)
    outT = nc.declare_dram_parameter("outT", [2, 64, S], F16, isOutput=True)

    with tile.TileContext(nc) as tc:
        with (
            tc.tile_pool(name="const", bufs=1) as const,
            tc.tile_pool(name="inbuf", bufs=1) as inbuf,
            tc.tile_pool(name="pbuf", bufs=6) as pbuf,
            tc.tile_pool(name="nbuf", bufs=2) as nbuf,
            tc.tile_pool(name="ps_s", bufs=2, space="PSUM") as ps_s,
            tc.tile_pool(name="ps_o", bufs=2, space="PSUM") as ps_o,
        ):

            # Input loads: few big DMAs. Only SP/gpsimd queues can issue DMAs
            # (Activation is the pacing engine - keep it clean). k0 + q3 gate
            # the first matmul; the rest stream in while span 3 computes.
            q0 = inbuf.tile([128, 512], F16, tag="q0", name="q0")
            nc.sync.dma_start(out=q0, in_=qT[:, 0:512])
            k0 = inbuf.tile([128, 512], F16, tag="k0", name="k0")
            nc.scalar.dma_start(out=k0, in_=kT[:, 0:512])
            q1 = inbuf.tile([128, 512], F16, tag="q1", name="q1")
            nc.sync.dma_start(out=q1, in_=qT[:, 512:1024])
            k1 = inbuf.tile([128, 512], F16, tag="k1", name="k1")
            nc.scalar.dma_start(out=k1, in_=kT[:, 512:1024])
            v0 = inbuf.tile([128, 65 * N_CHUNKS], BF16, tag="v0", name="v0")
            nc.sync.dma_start(out=v0, in_=vaug[0])
            k23 = inbuf.tile([128, 1024], F16, tag="k23", name="k23")
            nc.scalar.dma_start(out=k23, in_=kT[:, 1024:2048])
            v1 = inbuf.tile([128, 65 * N_CHUNKS], BF16, tag="v1", name="v1")
            nc.gpsimd.dma_start(out=v1, in_=vaug[1])
            q23 = inbuf.tile([128, 1024], F16, tag="q23", name="q23")
            nc.sync.dma_start(out=q23, in_=qT[:, 1024:2048])
            vsb = [v0, v1]

            mtri = const.tile([128, 128], BF16, tag="mtri", name="mtri")
            nc.gpsimd.memset(mtri, 1.0)
            nc.gpsimd.affine_select(
                out=mtri,
                in_=mtri,
                compare_op=mybir.AluOpType.is_ge,
                fill=0.0,
                base=0,
                pattern=[[1, 128]],
                channel_multiplier=-1,
            )

            def k_slice(h, kc):
                # kT chunk [64, 128] for head h: lhsT of the scores matmul.
                if kc < 4:
                    return k0[64 * h : 64 * h + 64, 128 * kc : 128 * kc + 128]
                if kc < 8:
                    c = 128 * kc - 512
                    return k1[64 * h : 64 * h + 64, c : c + 128]
                c = 128 * kc - 1024
                return k23[64 * h : 64 * h + 64, c : c + 128]

            def q_slice(h, qs, qe):
                if qe <= 512:
                    return q0[64 * h : 64 * h + 64, qs:qe]
                if qe <= 1024:
                    return q1[64 * h : 64 * h + 64, qs - 512 : qe - 512]
                return q23[64 * h : 64 * h + 64, qs - 1024 : qe - 1024]

            def v_slice(h, kc):
                return vsb[h][:, 65 * kc : 65 * kc + 65]

            def emit_span(s, po, flush_tails):
                qs, qe = 512 * s, 512 * (s + 1)
                # Tiles: non-diag chunks kc<4s get one [128,1024] tile each
                # (h0 block [0:512] bank A, h1 [512:1024] bank B). Diagonal
                # chunks are end-aligned: h0 at [512-w:512], h1 at [512:512+w],
                # so the exp covers 2w contiguous columns with no gap waste.
                # The two smallest diagonal chunks (w=256,128) share one tile.
                tiles = [[kc] for kc in range(4 * s)]
                tiles.append([4 * s])      # diag w=512 (full block, needs mask)
                tiles.append([4 * s + 1])  # diag w=384
                tiles.append([4 * s + 2, 4 * s + 3])  # diag w=256,128 merged
                for tix, kcs in enumerate(tiles):
                    if tix == 1:
                        flush_tails()
                    diag = kcs[0] >= 4 * s
                    pg = ps_s.tile([128, 1024], F32, tag="pss", name=f"pg_{s}_{kcs[0]}")
                    pe2 = pbuf.tile([128, 1024], BF16, tag="pe", name=f"pe_{s}_{kcs[0]}")
                    ws = [qe - max(qs, 128 * kc) for kc in kcs]
                    tot = sum(ws)
                    blocks = []  # (h, kc, off, w)
                    o = 512 - tot
                    for kc, w in reversed(list(zip(kcs, ws))):
                        blocks.append((0, kc, o, w))
                        o += w
                    o = 512
                    for kc, w in zip(kcs, ws):
                        blocks.append((1, kc, o, w))
                        o += w
                    lo, hi = 512 - tot, 512 + tot
                    # Score matmuls: per kc the (h0, h1) pair runs concurrently
                    # on PE row groups 0-63 / 64-127, draining into banks A/B.
                    order = sorted(blocks, key=lambda b: (b[1], b[0]))
                    for h, kc, off, w in order:
                        qb = qe - w
                        nc.tensor.matmul(
                            pg[:, off : off + w],
                            k_slice(h, kc),
                            q_slice(h, qb, qe),
                            start=True,
                            stop=True,
                        )
                    # One exp for the whole tile (both heads, all its chunks).
                    nc.scalar.activation(
                        out=pe2[:, lo:hi],
                        in_=pg[:, lo:hi],
                        func=mybir.ActivationFunctionType.Exp,
                        scale=0.125,
                    )
                    # Causal masks for diagonal chunks: zero the strict upper
                    # triangle of each block's first 128 query columns (DVE).
                    if diag:
                        for h, kc, off, w in order:
                            nc.vector.tensor_mul(
                                out=pe2[:, off : off + 128],
                                in0=pe2[:, off : off + 128],
                                in1=mtri,
                            )
                    # PV accumulation.
                    nkc = 4 * s + 4
                    for h, kc, off, w in order:
                        qb = qe - w
                        nc.tensor.matmul(
                            po[h][:, qb - qs : qb - qs + w],
                            v_slice(h, kc),
                            pe2[:, off : off + w],
                            start=(kc == 0),
                            stop=(kc == nkc - 1),
                        )

            def emit_tail(s, h, po, final=False):
                qs = 512 * s
                if final:
                    # final span's tails are the only exposed ones: pipeline
                    # in column halves with the copy on the now-idle scalar.
                    for hh in range(2):
                        dn = nbuf.tile([1, 512], F32, tag=f"dn{hh}", name=f"dnf{hh}")
                        r = nbuf.tile([1, 512], F32, tag=f"r{hh}", name=f"rf{hh}")
                        rb = nbuf.tile([64, 512], F32, tag=f"rb{hh}", name=f"rbf{hh}")
                        o_sb = nbuf.tile([64, 512], F16, tag=f"o{hh}", name=f"of{hh}")
                        for a, b in ((0, 256), (256, 512)):
                            nc.scalar.copy(out=dn[:, a:b], in_=po[hh][64:65, a:b])
                            nc.vector.reciprocal_approx_fast(
                                out=r[:, a:b], in_=dn[:, a:b]
                            )
                            nc.gpsimd.partition_broadcast(rb[:, a:b], r[0:1, a:b])
                            nc.vector.tensor_mul(
                                out=o_sb[:, a:b],
                                in0=po[hh][0:64, a:b],
                                in1=rb[:, a:b],
                            )
                            nc.sync.dma_start(
                                out=outT[hh, :, qs + a : qs + b], in_=o_sb[:, a:b]
                            )
                    return
                # 1/denom: copy the psum denominator row to a partition-0 SBUF
                # tile (custom DVE ops mis-read at nonzero partition offsets),
                # fast approx reciprocal (~51 ULP), broadcast across
                # partitions on the otherwise-idle gpsimd, normalize on DVE
                # (PSUM x SBUF -> f16), store via sync. The final span's tails
                # are the only exposed ones: pipeline them in column halves
                # with the copy on the (now idle) scalar engine.
                dn = nbuf.tile([1, 512], F32, tag=f"dn{h}", name=f"dn{h}_{s}")
                r = nbuf.tile([1, 512], F32, tag=f"r{h}", name=f"r{h}_{s}")
                rb = nbuf.tile([64, 512], F32, tag=f"rb{h}", name=f"rb{h}_{s}")
                o_sb = nbuf.tile([64, 512], F16, tag=f"o{h}", name=f"o{h}_{s}")
                nc.vector.tensor_copy(out=dn, in_=po[h][64:65, :])
                nc.vector.reciprocal_approx_fast(out=r, in_=dn)
                nc.gpsimd.partition_broadcast(rb[:, :], r[0:1, :])
                nc.vector.tensor_mul(out=o_sb, in0=po[h][0:64, :], in1=rb)
                nc.sync.dma_start(out=outT[h, :, 512 * s : 512 * (s + 1)], in_=o_sb)

            pending = []

            def flush_tails():
                while pending:
                    ps, ppo = pending.pop(0)
                    for h in range(2):
                        emit_tail(ps, h, ppo)

            for s in (0, 1, 2, 3):
                po = [
                    ps_o.tile([65, 512], F32, tag=f"po{hh}", name=f"po{hh}_{s}")
                    for hh in range(2)
                ]
                emit_span(s, po, flush_tails)
                pending.append((s, po))
            while pending:
                ps, ppo = pending.pop(0)
                emit_tail(ps, None, ppo, final=True)

    nc.compile()
    return nc


def _get_nc():
    global _NC
    if _NC is None:
        _NC = _build_bass()
    return _NC


def kernel(q, k, v):
    global _LAST_RESULTS
    q = np.asarray(q, dtype=np.float32)
    k = np.asarray(k, dtype=np.float32)
    v = np.asarray(v, dtype=np.float32)
    assert q.shape == (B, H, S, D)

    in_maps = []
    for c in range(N_CORES):
        h0 = HEADS_PER_CORE * c
        qTh = np.ascontiguousarray(
            q[0, h0 : h0 + 2].transpose(0, 2, 1).reshape(128, S)
        ).astype(np.float16)
        kTh = np.ascontiguousarray(
            k[0, h0 : h0 + 2].transpose(0, 2, 1).reshape(128, S)
        ).astype(np.float16)
        va = np.ones((2, 128, N_CHUNKS, 65), dtype=np.float32)
        va[..., :64] = (
            v[0, h0 : h0 + 2].reshape(2, N_CHUNKS, 128, 64).transpose(0, 2, 1, 3)
        )
        va16 = va.reshape(2, 128, 65 * N_CHUNKS).astype(ml_dtypes.bfloat16)
        in_maps.append({"qT": qTh, "kT": kTh, "vaug": va16})

    nc = _get_nc()
    res = run_bass_kernel_spmd(nc, in_maps, core_ids=list(range(N_CORES)))
    _LAST_RESULTS = res

    out = np.empty((B, H, S, D), dtype=np.float32)
    for c in range(N_CORES):
        ot = res.results[c]["outT"].astype(np.float32)  # [2, 64, 2048] f16
        out[0, 2 * c] = ot[0].T
        out[0, 2 * c + 1] = ot[1].T
    return out


# revision 20
# speedup vs baseline: 1.0291x; 1.0031x over previous
"""Causal flash attention for Trainium2, sharded 2 heads/core over 8 cores.

Math per head: out = softmax_causal(Q K^T / sqrt(D)) @ V,  Q/K/V [S=2048, D=64] fp32.

Device layout (per core, heads h0=2c, h1=2c+1):
  qT   [128, 2048]  rows 64h+d = Q[h]^T        (D on partitions, both heads stacked)
  kT   [128, 2048]  same for K
  vaug [2, 128, 1040] vaug[h, p, 65*kc+d] = V[h, 128*kc+p, d], d=64 column is ones
  outT [2, 64, 2048]  out[h]^T (normalized)

Scores are computed transposed (S^T[k, q] = K_chunk @ Q^T) so no transposes are
needed anywhere: softmax denominator comes out of the PV matmul via the ones
column of vaug (psum row 64), and the final division broadcasts 1/denom across
partitions with gpsimd.partition_broadcast.

The kernel is paced by the Scalar (ACT) engine's exp over every causal score
(34816 columns/core at 1 elem/cycle/lane, 1.2 GHz). Structure keeps ACT
saturated at that floor:
  - scores land in psum [128,1024] tiles (h0 in bank A, h1 in bank B; the
    head-pair matmuls run concurrently on PE row groups 0-63/64-127), one exp
    instruction per tile covering both heads.
  - diagonal chunks (width w < 512) are END-aligned against the bank boundary
    (h0 at [512-w:512], h1 at [512:512+w]) so the exp covers 2w contiguous
    columns with zero gap waste; the two smallest diagonal chunks of each span
    share one tile, giving 36 ACTIVATEs and exactly S^2/2-worth of columns.
  - causal triangle masks run on the DVE (multiply by a precomputed mask) so
    the Scalar queue carries nothing but the exps.
  - spans run smallest-first (0,1,2,3); each span's normalization tail is
    emitted one span late so every tail except the last hides under the next
    span's exp stream, and the last tail is pipelined in column halves with
    its denominator copies on the then-idle scalar engine.
  - inputs arrive as six large DMAs spread over the sync/scalar/gpsimd
    queues; outputs leave as fp16 (host converts back to fp32).

HW-verified pitfalls honored here: custom DVE ops (reciprocal_approx_fast)
mis-read operands at nonzero partition offsets and from PSUM (denominator row
is first copied to a partition-0 SBUF tile); matmul PSUM output must be fp32
on TRN2; DVE instructions may read at most one PSUM operand.
"""

import os
import sys

import ml_dtypes
import numpy as np

sys.path.insert(0, "/opt/trn_rl_repo")

import concourse.bass as bass
import concourse.bacc as bacc
import concourse.mybir as mybir
import concourse.tile as tile
from concourse.bass_utils import run_bass_kernel_spmd

B, H, S, D = 1, 16, 2048, 64
N_CORES = 8
HEADS_PER_CORE = H // N_CORES  # 2
N_CHUNKS = S // 128  # 16 key chunks per head
N_SPANS = S // 512  # 4 query spans per head
F32 = mybir.dt.float32
BF16 = mybir.dt.bfloat16
F16 = mybir.dt.float16

_NC = None
_LAST_RESULTS = None


def _build_bass():
    nc = bacc.Bacc("TRN2", target_bir_lowering=False)
    qT = nc.declare_dram_parameter("qT", [128, S], F16, isOutput=False)
    kT = nc.declare_dram_parameter("kT", [128, S], F16, isOutput=False)
    vaug = nc.declare_dram_parameter("vaug", [2, 128, 65 * N_CHUNKS], BF16, isOutput=False)
    outT = nc.declare_dram_parameter("outT", [2, 64, S], F16, isOutput=True)

    with tile.TileContext(nc) as tc:
        with (
            tc.tile_pool(name="const", bufs=1) as const,
            tc.tile_pool(name="inbuf", bufs=1) as inbuf,
            tc.tile_pool(name="pbuf", bufs=6) as pbuf,
            tc.tile_pool(name="nbuf", bufs=2) as nbuf,
            tc.tile_pool(name="ps_s", bufs=2, space="PSUM") as ps_s,
            tc.tile_pool(name="ps_o", bufs=2, space="PSUM") as ps_o,
        ):

            # Input loads: few big DMAs. Only SP/gpsimd queues can issue DMAs
            # (Activation is the pacing engine - keep it clean). k0 + q3 gate
            # the first matmul; the rest stream in while span 3 computes.
            q0 = inbuf.tile([128, 512], F16, tag="q0", name="q0")
            nc.sync.dma_start(out=q0, in_=qT[:, 0:512])
            k0 = inbuf.tile([128, 512], F16, tag="k0", name="k0")
            nc.scalar.dma_start(out=k0, in_=kT[:, 0:512])
            q1 = inbuf.tile([128, 512], F16, tag="q1", name="q1")
            nc.sync.dma_start(out=q1, in_=qT[:, 512:1024])
            k1 = inbuf.tile([128, 512], F16, tag="k1", name="k1")
            nc.scalar.dma_start(out=k1, in_=kT[:, 512:1024])
            v0 = inbuf.tile([128, 65 * N_CHUNKS], BF16, tag="v0", name="v0")
            nc.sync.dma_start(out=v0, in_=vaug[0])
            k23 = inbuf.tile([128, 1024], F16, tag="k23", name="k23")
            nc.scalar.dma_start(out=k23, in_=kT[:, 1024:2048])
            v1 = inbuf.tile([128, 65 * N_CHUNKS], BF16, tag="v1", name="v1")
            nc.gpsimd.dma_start(out=v1, in_=vaug[1])
            q23 = inbuf.tile([128, 1024], F16, tag="q23", name="q23")
            nc.sync.dma_start(out=q23, in_=qT[:, 1024:2048])
            vsb = [v0, v1]

            mtri = const.tile([128, 128], BF16, tag="mtri", name="mtri")
            nc.gpsimd.memset(mtri, 1.0)
            nc.gpsimd.affine_select(
                out=mtri,
                in_=mtri,
                compare_op=mybir.AluOpType.is_ge,
                fill=0.0,
                base=0,
                pattern=[[1, 128]],
                channel_multiplier=-1,
            )

            def k_slice(h, kc):
                # kT chunk [64, 128] for head h: lhsT of the scores matmul.
                if kc < 4:
                    return k0[64 * h : 64 * h + 64, 128 * kc : 128 * kc + 128]
                if kc < 8:
                    c = 128 * kc - 512
                    return k1[64 * h : 64 * h + 64, c : c + 128]
                c = 128 * kc - 1024
                return k23[64 * h : 64 * h + 64, c : c + 128]

            def q_slice(h, qs, qe):
                if qe <= 512:
                    return q0[64 * h : 64 * h + 64, qs:qe]
                if qe <= 1024:
                    return q1[64 * h : 64 * h + 64, qs - 512 : qe - 512]
                return q23[64 * h : 64 * h + 64, qs - 1024 : qe - 1024]

            def v_slice(h, kc):
                return vsb[h][:, 65 * kc : 65 * kc + 65]

            def emit_span(s, po, flush_tails):
                qs, qe = 512 * s, 512 * (s + 1)
                # Tiles: non-diag chunks kc<4s get one [128,1024] tile each
                # (h0 block [0:512] bank A, h1 [512:1024] bank B). Diagonal
                # chunks are end-aligned: h0 at [512-w:512], h1 at [512:512+w],
                # so the exp covers 2w contiguous columns with no gap waste.
                # The two smallest diagonal chunks (w=256,128) share one tile.
                tiles = [[kc] for kc in range(4 * s)]
                tiles.append([4 * s])      # diag w=512 (full block, needs mask)
                tiles.append([4 * s + 1])  # diag w=384
                tiles.append([4 * s + 2, 4 * s + 3])  # diag w=256,128 merged
                for tix, kcs in enumerate(tiles):
                    if tix == 1:
                        flush_tails()
                    diag = kcs[0] >= 4 * s
                    pg = ps_s.tile([128, 1024], F32, tag="pss", name=f"pg_{s}_{kcs[0]}")
                    pe2 = pbuf.tile([128, 1024], BF16, tag="pe", name=f"pe_{s}_{kcs[0]}")
                    ws = [qe - max(qs, 128 * kc) for kc in kcs]
                    tot = sum(ws)
                    blocks = []  # (h, kc, off, w)
                    o = 512 - tot
                    for kc, w in reversed(list(zip(kcs, ws))):
                        blocks.append((0, kc, o, w))
                        o += w
                    o = 512
                    for kc, w in zip(kcs, ws):
                        blocks.append((1, kc, o, w))
                        o += w
                    lo, hi = 512 - tot, 512 + tot
                    # Score matmuls: per kc the (h0, h1) pair runs concurrently
                    # on PE row groups 0-63 / 64-127, draining into banks A/B.
                    order = sorted(blocks, key=lambda b: (b[1], b[0]))
                    for h, kc, off, w in order:
                        qb = qe - w
                        nc.tensor.matmul(
                            pg[:, off : off + w],
                            k_slice(h, kc),
                            q_slice(h, qb, qe),
                            start=True,
                            stop=True,
                        )
                    # One exp for the whole tile (both heads, all its chunks).
                    nc.scalar.activation(
                        out=pe2[:, lo:hi],
                        in_=pg[:, lo:hi],
                        func=mybir.ActivationFunctionType.Exp,
                        scale=0.125,
                    )
                    # Causal masks for diagonal chunks: zero the strict upper
                    # triangle of each block's first 128 query columns (DVE).
                    if diag:
                        for h, kc, off, w in order:
                            nc.vector.tensor_mul(
                                out=pe2[:, off : off + 128],
                                in0=pe2[:, off : off + 128],
                                in1=mtri,
                            )
                    # PV accumulation.
                    nkc = 4 * s + 4
                    for h, kc, off, w in order:
                        qb = qe - w
                        nc.tensor.matmul(
                            po[h][:, qb - qs : qb - qs + w],
                            v_slice(h, kc),
                            pe2[:, off : off + w],
                            start=(kc == 0),
                            stop=(kc == nkc - 1),
                        )

            def emit_tail(s, h, po, final=False):
                qs = 512 * s
                if final:
                    # final span's tails are the only exposed ones: pipeline
                    # in column halves with the copy on the now-idle scalar.
                    for hh in range(2):
                        dn = nbuf.tile([1, 512], F32, tag=f"dn{hh}", name=f"dnf{hh}")
                        r = nbuf.tile([1, 512], F32, tag=f"r{hh}", name=f"rf{hh}")
                        rb = nbuf.tile([64, 512], F32, tag=f"rb{hh}", name=f"rbf{hh}")
                        o_sb = nbuf.tile([64, 512], F16, tag=f"o{hh}", name=f"of{hh}")
                        for a, b in ((0, 256), (256, 512)):
                            nc.scalar.copy(out=dn[:, a:b], in_=po[hh][64:65, a:b])
                            nc.vector.reciprocal_approx_fast(
                                out=r[:, a:b], in_=dn[:, a:b]
                            )
                            nc.gpsimd.partition_broadcast(rb[:, a:b], r[0:1, a:b])
                            nc.vector.tensor_mul(
                                out=o_sb[:, a:b],
                                in0=po[hh][0:64, a:b],
                                in1=rb[:, a:b],
                            )
                            nc.sync.dma_start(
                                out=outT[hh, :, qs + a : qs + b], in_=o_sb[:, a:b]
                            )
                    return
                # 1/denom: copy the psum denominator row to a partition-0 SBUF
                # tile (custom DVE ops mis-read at nonzero partition offsets),
                # fast approx reciprocal (~51 ULP), broadcast across
                # partitions on the otherwise-idle gpsimd, normalize on DVE
                # (PSUM x SBUF -> f16), store via sync. The final span's tails
                # are the only exposed ones: pipeline them in column halves
                # with the copy on the (now idle) scalar engine.
                dn = nbuf.tile([1, 512], F32, tag=f"dn{h}", name=f"dn{h}_{s}")
                r = nbuf.tile([1, 512], F32, tag=f"r{h}", name=f"r{h}_{s}")
                rb = nbuf.tile([64, 512], F32, tag=f"rb{h}", name=f"rb{h}_{s}")
                o_sb = nbuf.tile([64, 512], F16, tag=f"o{h}", name=f"o{h}_{s}")
                nc.vector.tensor_copy(out=dn, in_=po[h][64:65, :])
                nc.vector.reciprocal_approx_fast(out=r, in_=dn)
                nc.gpsimd.partition_broadcast(rb[:, :], r[0:1, :])
                nc.vector.tensor_mul(out=o_sb, in0=po[h][0:64, :], in1=rb)
                nc.sync.dma_start(out=outT[h, :, 512 * s : 512 * (s + 1)], in_=o_sb)

            pending = []

            def flush_tails():
                while pending:
                    ps, ppo = pending.pop(0)
                    for h in range(2):
                        emit_tail(ps, h, ppo)

            for s in (0, 1, 2, 3):
                po = [
                    ps_o.tile([65, 512], F32, tag=f"po{hh}", name=f"po{hh}_{s}")
                    for hh in range(2)
                ]
                emit_span(s, po, flush_tails)
                pending.append((s, po))
            while pending:
                ps, ppo = pending.pop(0)
                emit_tail(ps, None, ppo, final=True)

    nc.compile()
    return nc


def _get_nc():
    global _NC
    if _NC is None:
        _NC = _build_bass()
    return _NC


def kernel(q, k, v):
    global _LAST_RESULTS
    q = np.asarray(q, dtype=np.float32)
    k = np.asarray(k, dtype=np.float32)
    v = np.asarray(v, dtype=np.float32)
    assert q.shape == (B, H, S, D)

    in_maps = []
    for c in range(N_CORES):
        h0 = HEADS_PER_CORE * c
        qTh = np.ascontiguousarray(
            q[0, h0 : h0 + 2].transpose(0, 2, 1).reshape(128, S)
        ).astype(np.float16)
        kTh = np.ascontiguousarray(
            k[0, h0 : h0 + 2].transpose(0, 2, 1).reshape(128, S)
        ).astype(np.float16)
        va = np.ones((2, 128, N_CHUNKS, 65), dtype=np.float32)
        va[..., :64] = (
            v[0, h0 : h0 + 2].reshape(2, N_CHUNKS, 128, 64).transpose(0, 2, 1, 3)
        )
        va16 = va.reshape(2, 128, 65 * N_CHUNKS).astype(ml_dtypes.bfloat16)
        in_maps.append({"qT": qTh, "kT": kTh, "vaug": va16})

    nc = _get_nc()
    res = run_bass_kernel_spmd(nc, in_maps, core_ids=list(range(N_CORES)))
    _LAST_RESULTS = res

    out = np.empty((B, H, S, D), dtype=np.float32)
    for c in range(N_CORES):
        ot = res.results[c]["outT"].astype(np.float32)  # [2, 64, 2048] f16
        out[0, 2 * c] = ot[0].T
        out[0, 2 * c + 1] = ot[1].T
    return out
